# revision 17
# baseline (speedup 1.0000x reference)
"""BertSelfAttention (B=4, S=4096, D=512) on 8 TRN2 NeuronCores.

Sharding: core c handles batch b = c//2 and query-row half h = c%2
(2048 q rows). K/V are computed on-core for the full 4096 keys of that
batch (halves exchanged within each core pair), avoiding big collectives.

Layout trick: everything is computed transposed so no on-device
transposes are needed:
  QT[e, q] = Wq @ x.T          (lhsT = WqT chunks, rhs = xT chunks)
  KT[e, k] = Wk @ x.T
  V [k, e] = x @ Wv.T          (lhsT = xT chunks,  rhs = WvT)
  ST[k, q] = K Q.T             (lhsT = KT chunks,  rhs = QT)   -> exp -> PT
  OT[e, q] = V.T P.T           (lhsT = V chunks,   rhs = PT)
Softmax runs without max-subtraction (scores are ~N(0, 0.3^2), so exp
cannot overflow and the result is mathematically identical).

Precision: projections run in bf16 (fp32 PSUM); Q/K/V/P are quantized
to fp8e4 (e4m3) on PSUM evacuation and the two big matmuls (scores and
attn@V) run as fp8 DoubleRow matmuls: lhsT [128, 2, 128] and rhs
[128, 2, 512] slices pair two 128-deep contraction chunks per pass,
halving PE cycles per MAC. The dominant quantization-error term,
P @ (V - Vq), has a rank-1 component: softmax weights sum to exactly 1
(we normalize by the row sums of the quantized P), so the key-mean of
the V quantization error adds the same vector to every output row.
Each core accumulates column sums of (V_f32 - V_fp8) for its local
half on the otherwise-idle Vector engine, partition-reduces them with
a ones-matmul, and a tiny third AllGather ([1, 512] f32) shares them
across the pair; the correction folds into the bv bias applied during
output evacuation. Row sums of P are accumulated on the Vector engine
(one add per exp'd tile, pipelined behind the Scalar-engine exps),
then reduced across partitions and replicated by a single all-ones
matmul per q-chunk. Normalization + bias happen during OT evacuation
(exact: P@(V + 1*bv)/rowsum = P@V/rowsum + bv). A burst of throwaway
matmuls warms the PE HAM clock gate during the initial DMA wait, and
input DMAs are column-chunked and issued in first-consumer order on
the SP hardware queue (weights on the ACT queue) so the first
projection starts after ~1 MB of transfer.
"""

import sys

for _p in ("/opt/trn_rl_repo", "/root/.axon_site/_ro/trn_rl_repo"):
    if _p not in sys.path:
        sys.path.append(_p)

import numpy as np
import ml_dtypes

B, S, D = 4, 4096, 512
NCORES = 8
SQ = S // 2  # query rows per core
P = 128
NQ = 512  # q-chunk width (moving free dim)
DT = D // P  # 4 contraction chunks for d
ET = D // P  # 4 e tiles
KTI = S // P  # 32 k tiles
QC = SQ // NQ  # 4 q chunks per core
KC = S // NQ  # 8 k chunks (KT projection)
HKT = KTI // 2  # 16 local k-tiles per core
HS = S // 2  # 2048 local keys per core
SCALE = 1.0 / float(np.sqrt(np.float32(D)))

_CACHE = {}


def _split_excess_waits(nc, mybir, max_waits=1):
    """This walrus build rejects instructions carrying more than a couple of
    sync waits. Cap every instruction at `max_waits`, spilling the rest onto
    same-engine InstNoOps inserted immediately before it (equivalent
    semantics: the engine's stream stalls at the nop instead)."""
    for f in nc.m.functions:
        for bb in f.blocks:
            old = list(bb.instructions)
            new = []
            for inst in old:
                si = inst.sync_info
                waits = list(si.on_wait) if si is not None and si.on_wait else []
                if len(waits) > max_waits:
                    keep = waits[-max_waits:]
                    excess = waits[:-max_waits]
                    for i in range(0, len(excess), max_waits):
                        nop = mybir.InstNoOp(
                            name=f"waitnop-{nc.next_id()}", ins=[], outs=[]
                        )
                        nop.engine = inst.engine
                        nop.sync_info = mybir.SyncInfo(
                            on_wait=excess[i : i + max_waits], on_update=[]
                        )
                        new.append(nop)
                    inst.sync_info = mybir.SyncInfo(
                        on_wait=keep,
                        on_update=list(si.on_update) if si.on_update else [],
                    )
                new.append(inst)
            if len(new) != len(old):
                bb.instructions[:] = new


def _build_nc():
    import concourse.bass as bass
    import concourse.mybir as mybir
    import concourse.tile as tile
    from contextlib import ExitStack

    bf = mybir.dt.bfloat16
    f32 = mybir.dt.float32
    f8 = mybir.dt.float8e4
    AF = mybir.ActivationFunctionType
    DR = mybir.MatmulPerfMode.DoubleRow

    u32 = mybir.dt.uint32
    nc = bass.Bass()
    xT = nc.declare_dram_parameter("xT", [D, SQ], bf, isOutput=False)
    wqT = nc.declare_dram_parameter("wqT", [D, D], bf, isOutput=False)
    wkT = nc.declare_dram_parameter("wkT", [D, D], bf, isOutput=False)
    wvT = nc.declare_dram_parameter("wvT", [D, D], bf, isOutput=False)
    bqp = nc.declare_dram_parameter("bq", [P, ET], f32, isOutput=False)
    bkp = nc.declare_dram_parameter("bk", [P, ET], f32, isOutput=False)
    bvp = nc.declare_dram_parameter("bv", [P, ET], f32, isOutput=False)
    # Host-computed row bases into the AllGather outputs for the PARTNER
    # half (rank-dependent: (1-h)*512 + e*128 for KT, (1-h)*2048 + j*128
    # for V). Drives dynamic (register-offset) DMAs.
    poffp = nc.declare_dram_parameter("poff", [1, 2], u32, isOutput=False)
    ot = nc.declare_dram_parameter("ot", [D, SQ], f32, isOutput=True)

    with tile.TileContext(nc) as tc, ExitStack() as ctx:
        const_pool = ctx.enter_context(tc.tile_pool(name="const", bufs=1))
        persist = ctx.enter_context(tc.tile_pool(name="persist", bufs=1))
        outp = ctx.enter_context(tc.tile_pool(name="outp", bufs=2))

        ones = const_pool.tile([P, P], bf, tag="ones")
        nc.vector.memset(ones, 1.0)
        bq_sb = const_pool.tile([P, ET], f32, tag="bq")
        bk_sb = const_pool.tile([P, ET], f32, tag="bk")
        bv_sb = const_pool.tile([P, ET], f32, tag="bv")
        wq_sb = [const_pool.tile([P, D], bf, tag=f"wq{d}", name=f"wq{d}") for d in range(DT)]
        wk_sb = [const_pool.tile([P, D], bf, tag=f"wk{d}", name=f"wk{d}") for d in range(DT)]
        wv_sb = [const_pool.tile([P, D], bf, tag=f"wv{d}", name=f"wv{d}") for d in range(DT)]
        # fp8 operand tiles for the DoubleRow matmuls; contraction-paired
        # chunks live in dim 1 so [:, e:e+2, cols] is a valid 3D AP.
        qt_sb = persist.tile([P, ET, SQ], f8, tag="qt", name="qt")
        # K/V k-order per core: [my half, partner half]. Separate tiles per
        # half so partner DMA-writes create no false deps on local reads.
        kt_loc = persist.tile([P, ET, HS], f8, tag="ktl", name="ktl")
        kt_rem = persist.tile([P, ET, HS], f8, tag="ktr", name="ktr")
        v_loc = persist.tile([P, HKT, D], f8, tag="vl", name="vl")
        v_rem = persist.tile([P, HKT, D], f8, tag="vr", name="vr")
        poff_sb = const_pool.tile([1, 2], mybir.dt.uint32, tag="poff")
        bias2 = const_pool.tile([P, ET], f32, tag="bias2")

        # ---- Phase 1+2: load local x, project Q + local-half K/V,
        # AllGather the K/V halves within each core pair ----
        with (
            tc.tile_pool(name="xin", bufs=1) as xin_pool,
            tc.tile_pool(name="psA", bufs=4, space="PSUM") as psA,
            tc.tile_pool(name="dram", bufs=1, space="DRAM") as dram,
        ):
            ktl_d = dram.tile([ET * P, HS], f8, tag="ktl_d")
            ktg_d = dram.tile([2 * ET * P, HS], f8, tag="ktg_d")
            vl_d = dram.tile([HKT * P, D], f8, tag="vl_d")
            vg_d = dram.tile([2 * HKT * P, D], f8, tag="vg_d")
            sl_d = dram.tile([1, D], f32, tag="sl_d")
            sg_d = dram.tile([2, D], f32, tag="sg_d")

            x_sb = [xin_pool.tile([P, HS], bf, tag=f"x{d}", name=f"x{d}") for d in range(DT)]
            # Column-chunked loads in first-consumer order (SP HW queue);
            # weights ride the ACT HW queue in parallel.
            # One narrow chunk first (fast start for the first KT groups),
            # then the remaining columns in one wide DMA per d-tile to keep
            # SP-side issue overhead (~0.6us per DMA) off the critical path.
            for d in range(DT):
                nc.sync.dma_start(
                    out=x_sb[d][:, :NQ], in_=xT[d * P : (d + 1) * P, :NQ]
                )
            nc.sync.dma_start(out=bk_sb, in_=bkp[:, :])
            nc.sync.dma_start(out=bq_sb, in_=bqp[:, :])
            nc.sync.dma_start(out=bv_sb, in_=bvp[:, :])
            nc.sync.dma_start(out=poff_sb, in_=poffp[:, :])
            for kc in range(1, QC):
                for d in range(DT):
                    nc.sync.dma_start(
                        out=x_sb[d][:, kc * NQ : (kc + 1) * NQ],
                        in_=xT[d * P : (d + 1) * P, kc * NQ : (kc + 1) * NQ],
                    )
            for d in range(DT):
                nc.scalar.dma_start(out=wk_sb[d], in_=wkT[d * P : (d + 1) * P, :])
            for d in range(DT):
                nc.scalar.dma_start(out=wv_sb[d], in_=wvT[d * P : (d + 1) * P, :])
            for d in range(DT):
                nc.scalar.dma_start(out=wq_sb[d], in_=wqT[d * P : (d + 1) * P, :])

            # Warm the PE HAM clock gate (~3.4us of activity flips it from
            # 1.2 to 2.4 GHz) with throwaway matmuls while the first input
            # DMAs are still in flight.
            warm_ps = psA.tile([P, P], f32, tag="warm", name="warm_ps", bufs=1)
            for _ in range(96):
                nc.tensor.matmul(warm_ps, lhsT=ones, rhs=ones, start=True, stop=True)

            # KT local half [e, 0:2048] (bias bk fused on evacuation)
            for kc in range(QC):
                for e in range(ET):
                    ps = psA.tile([P, NQ], f32, tag="ps")
                    for d in range(DT):
                        nc.tensor.matmul(
                            ps,
                            lhsT=wk_sb[d][:, e * P : (e + 1) * P],
                            rhs=x_sb[d][:, kc * NQ : (kc + 1) * NQ],
                            start=(d == 0),
                            stop=(d == DT - 1),
                        )
                    nc.scalar.activation(
                        out=kt_loc[:, e, kc * NQ : (kc + 1) * NQ],
                        in_=ps,
                        func=AF.Identity,
                        bias=bk_sb[:, e : e + 1],
                        scale=1.0,
                    )
            for e in range(ET):
                nc.sync.dma_start(out=ktl_d[e * P : (e + 1) * P, :], in_=kt_loc[:, e, :])
            # Start the KT exchange immediately: the CC engine is idle and
            # the partner half gates the second half of the score phase.
            pairs = [[2 * i, 2 * i + 1] for i in range(NCORES // 2)]
            nc.gpsimd.collective_compute(
                "AllGather",
                mybir.AluOpType.bypass,
                replica_groups=pairs,
                ins=[ktl_d.opt()],
                outs=[ktg_d.opt()],
            )
            # V local half, tiles 0..15 (no bias; bv folded in at the end).
            # The Vector engine accumulates column sums of the fp8
            # quantization error for the rank-1 output correction.
            dacc = outp.tile([P, D], f32, tag="dacc", bufs=1)
            nc.vector.memset(dacc, 0.0)
            for k in range(HKT):
                ps = psA.tile([P, D], f32, tag="ps")
                for d in range(DT):
                    nc.tensor.matmul(
                        ps,
                        lhsT=x_sb[d][:, k * P : (k + 1) * P],
                        rhs=wv_sb[d][:, :],
                        start=(d == 0),
                        stop=(d == DT - 1),
                    )
                nc.scalar.copy(out=v_loc[:, k, :], in_=ps)
                dtmp = outp.tile([P, D], f32, tag="dtmp", bufs=2)
                nc.vector.tensor_sub(dtmp, ps, v_loc[:, k, :])
                nc.vector.tensor_add(dacc, dacc, dtmp)
                nc.sync.dma_start(out=vl_d[k * P : (k + 1) * P, :], in_=v_loc[:, k, :])
            # Partition-reduce the dV column sums and publish for the pair.
            dsum_bf = outp.tile([P, D], bf, tag="dsum_bf", bufs=1)
            nc.vector.tensor_copy(out=dsum_bf, in_=dacc)
            ps1 = psA.tile([1, D], f32, tag="ps1", bufs=1)
            nc.tensor.matmul(ps1, lhsT=ones[:, 0:1], rhs=dsum_bf, start=True, stop=True)
            sl_sb = outp.tile([1, D], f32, tag="sl_sb", bufs=1)
            nc.scalar.copy(out=sl_sb, in_=ps1)
            nc.sync.dma_start(out=sl_d, in_=sl_sb)

            # Exchange the V half + dV column sums while QT + scores run.
            nc.gpsimd.collective_compute(
                "AllGather",
                mybir.AluOpType.bypass,
                replica_groups=pairs,
                ins=[vl_d.opt()],
                outs=[vg_d.opt()],
            )
            nc.gpsimd.collective_compute(
                "AllGather",
                mybir.AluOpType.bypass,
                replica_groups=pairs,
                ins=[sl_d.opt()],
                outs=[sg_d.opt()],
            )

            # Partner-half loads from the gather outputs, issued before the
            # QT projection so the transfers start the moment each gather
            # lands. The row base is rank-dependent, supplied by the host
            # via `poff` and applied as a dynamic (register) offset.
            SP = [mybir.EngineType.SP]
            kt_base = nc.values_load(
                poff_sb[0:1, 0:1], engines=SP,
                min_val=0, max_val=ET * P,
                skip_runtime_bounds_check=True,
            )
            nc.sync.dma_start(
                out=kt_rem,
                in_=ktg_d[bass.ds(kt_base, ET * P), :].rearrange(
                    "(e p) c -> p e c", p=P
                ),
            )
            v_base = nc.values_load(
                poff_sb[0:1, 1:2], engines=SP,
                min_val=0, max_val=HKT * P,
                skip_runtime_bounds_check=True,
            )
            nc.sync.dma_start(
                out=v_rem,
                in_=vg_d[bass.ds(v_base, HKT * P), :].rearrange(
                    "(j p) c -> p j c", p=P
                ),
            )
            s_a = outp.tile([P, ET], f32, tag="s_a", bufs=1)
            s_b = outp.tile([P, ET], f32, tag="s_b", bufs=1)
            nc.sync.dma_start(
                out=s_a, in_=sg_d[0:1, :].rearrange("r (et p) -> (r p) et", p=P)
            )
            nc.sync.dma_start(
                out=s_b, in_=sg_d[1:2, :].rearrange("r (et p) -> (r p) et", p=P)
            )

            # QT[e, q] (bias bq fused on evacuation)
            for qc in range(QC):
                for e in range(ET):
                    ps = psA.tile([P, NQ], f32, tag="ps")
                    for d in range(DT):
                        nc.tensor.matmul(
                            ps,
                            lhsT=wq_sb[d][:, e * P : (e + 1) * P],
                            rhs=x_sb[d][:, qc * NQ : (qc + 1) * NQ],
                            start=(d == 0),
                            stop=(d == DT - 1),
                        )
                    nc.vector.tensor_scalar_add(
                        out=qt_sb[:, e, qc * NQ : (qc + 1) * NQ],
                        in0=ps,
                        scalar1=bq_sb[:, e : e + 1],
                    )

        # ---- Phase 3: attention ----
        # Static emission order staggers local-half score blocks ahead of
        # partner-half blocks so the PE has work while the AllGather +
        # partner DMAs are in flight.
        with (
            tc.tile_pool(name="pt", bufs=1) as pt_pool,
            tc.tile_pool(name="ps_st", bufs=3, space="PSUM") as ps_st,
            tc.tile_pool(name="ps_ot", bufs=2, space="PSUM") as ps_ot,
        ):
            ptl_tiles = {}
            ptp_tiles = {}
            rs_accs = {}
            rs_gps = {}

            def pt_slice(qc, k):
                if k < HKT:
                    return ptl_tiles[qc][:, k, :]
                return ptp_tiles[qc][:, k - HKT, :]

            def st_block(qc, k0, k1):
                qsl = slice(qc * NQ, (qc + 1) * NQ)
                if k0 == 0:
                    ptl_tiles[qc] = pt_pool.tile(
                        [P, HKT, NQ], f8, tag="ptl", name=f"ptl{qc}", bufs=4
                    )
                else:
                    ptp_tiles[qc] = pt_pool.tile(
                        [P, HKT, NQ], f8, tag="ptp", name=f"ptp{qc}", bufs=3
                    )
                # Two k-tiles share one 2-bank PSUM tile so each ACT Exp
                # covers 1024 columns, halving the per-instruction overhead.
                for k in range(k0, k1, 2):
                    ps = ps_st.tile([P, 2, NQ], f32, tag="st", name="st_ps")
                    for kh in range(2):
                        for e in (0, 2):
                            if k < HKT:
                                lhsT = kt_loc[:, e : e + 2, (k + kh) * P : (k + kh + 1) * P]
                            else:
                                kk = k + kh - HKT
                                lhsT = kt_rem[:, e : e + 2, kk * P : (kk + 1) * P]
                            nc.tensor.matmul(
                                ps[:, kh, :],
                                lhsT=lhsT,
                                rhs=qt_sb[:, e : e + 2, qsl],
                                start=(e == 0),
                                stop=(e == 2),
                                perf_mode=DR,
                            )
                    if k < HKT:
                        pt_pair = ptl_tiles[qc][:, k : k + 2, :]
                    else:
                        pt_pair = ptp_tiles[qc][:, k - HKT : k - HKT + 2, :]
                    nc.scalar.activation(
                        out=pt_pair, in_=ps, func=AF.Exp, scale=SCALE
                    )
                    if k == 0:
                        rs_accs[qc] = outp.tile(
                            [P, NQ], f32, tag="rs_acc", name=f"rs_acc{qc}", bufs=4
                        )
                        rs_gps[qc] = outp.tile(
                            [P, NQ], f32, tag="rs_gp", name=f"rs_gp{qc}", bufs=4
                        )
                        nc.vector.tensor_copy(out=rs_accs[qc], in_=pt_slice(qc, 0))
                        nc.gpsimd.tensor_copy(out=rs_gps[qc], in_=pt_slice(qc, 1))
                    else:
                        nc.vector.tensor_add(
                            rs_accs[qc], rs_accs[qc], pt_slice(qc, k)
                        )
                        nc.gpsimd.tensor_add(
                            rs_gps[qc], rs_gps[qc], pt_slice(qc, k + 1)
                        )

            recips = {}

            def bias2_compute():
                # dV column sums from both halves -> output bias correction
                # bias2 = bv + (s_local + s_partner) / S. Rank order in sg_d
                # is irrelevant since both rows are summed. Emitted here (at
                # first consumption) so the in-order ACT stream never stalls
                # on the small s-gather.
                nc.vector.tensor_add(s_a, s_a, s_b)
                for et in range(ET):
                    nc.scalar.activation(
                        out=bias2[:, et : et + 1],
                        in_=s_a[:, et : et + 1],
                        func=AF.Identity,
                        bias=bv_sb[:, et : et + 1],
                        scale=1.0 / S,
                    )

            rs_bfs = {}

            def finish_rsbf(qc):
                # ACT-side bf16 copy of the rowsum partials, emitted right
                # after st_block(qc, partner) so it sits directly behind that
                # block's exps in the in-order ACT stream.
                nc.vector.tensor_add(rs_accs[qc], rs_accs[qc], rs_gps[qc])
                rs_bfs[qc] = outp.tile(
                    [P, NQ], bf, tag="rs_bf", bufs=2, name=f"rs_bf{qc}"
                )
                nc.scalar.copy(out=rs_bfs[qc], in_=rs_accs[qc])

            def finish_rsmm(qc):
                # Partition-reduce + replicate the rowsum partials with one
                # all-ones matmul. Deferred one PE block after finish_rsbf so
                # the PE never leads the ACT copy it depends on.
                rs_ps = ps_ot.tile([P, NQ], f32, tag="ot", name="rs_ps")
                nc.tensor.matmul(rs_ps, lhsT=ones, rhs=rs_bfs[qc], start=True, stop=True)
                recips[qc] = outp.tile([P, NQ], f32, tag="recip", bufs=2, name=f"recip{qc}")
                nc.vector.reciprocal(recips[qc], rs_ps)

            def finish_av(qc):
                qsl = slice(qc * NQ, (qc + 1) * NQ)
                recip = recips[qc]
                for e in range(ET):
                    ops = ps_ot.tile([P, NQ], f32, tag="ot")
                    for k in range(0, KTI, 2):
                        if k < HKT:
                            vlhsT = v_loc[:, k : k + 2, e * P : (e + 1) * P]
                            prhs = ptl_tiles[qc][:, k : k + 2, :]
                        else:
                            kk = k - HKT
                            vlhsT = v_rem[:, kk : kk + 2, e * P : (e + 1) * P]
                            prhs = ptp_tiles[qc][:, kk : kk + 2, :]
                        nc.tensor.matmul(
                            ops,
                            lhsT=vlhsT,
                            rhs=prhs,
                            start=(k == 0),
                            stop=(k == KTI - 2),
                            perf_mode=DR,
                        )
                    tmp = outp.tile([P, NQ], f32, tag="tmp", bufs=3)
                    nc.vector.tensor_mul(tmp, ops, recip)
                    nc.scalar.activation(
                        out=tmp,
                        in_=tmp,
                        func=AF.Identity,
                        bias=bias2[:, e : e + 1],
                        scale=1.0,
                    )
                    nc.sync.dma_start(out=ot[e * P : (e + 1) * P, qsl], in_=tmp)

            # Interleave: each q-chunk's rowsum reduction is emitted right
            # after its last score block (ahead of the next block's exps in
            # the ACT stream), and its AV matmuls run while the following
            # block's exp/rowsum tail resolves on ACT/DVE.
            st_block(0, 0, HKT)
            st_block(1, 0, HKT)
            st_block(2, 0, HKT)
            st_block(3, 0, HKT)
            st_block(0, HKT, KTI)
            finish_rsbf(0)
            st_block(1, HKT, KTI)
            finish_rsbf(1)
            finish_rsmm(0)
            bias2_compute()
            finish_av(0)
            st_block(2, HKT, KTI)
            finish_rsbf(2)
            finish_rsmm(1)
            finish_av(1)
            st_block(3, HKT, KTI)
            finish_rsbf(3)
            finish_rsmm(2)
            finish_av(2)
            finish_rsmm(3)
            finish_av(3)

    _split_excess_waits(nc, mybir)
    return nc


def _get_nc():
    if "nc" not in _CACHE:
        _CACHE["nc"] = _build_nc()
    return _CACHE["nc"]


def _make_in_maps(x, Wq, bq, Wk, bk, Wv, bv):
    bf16 = ml_dtypes.bfloat16
    wqT = np.ascontiguousarray(Wq.T).astype(bf16)
    wkT = np.ascontiguousarray(Wk.T).astype(bf16)
    wvT = np.ascontiguousarray(Wv.T).astype(bf16)
    bqp = np.ascontiguousarray(bq.reshape(ET, P).T).astype(np.float32)
    bkp = np.ascontiguousarray(bk.reshape(ET, P).T).astype(np.float32)
    bvp = np.ascontiguousarray(bv.reshape(ET, P).T).astype(np.float32)
    in_maps = []
    for c in range(NCORES):
        b, h = divmod(c, 2)
        # Local half of x[b].T: both this core's query columns and its K/V
        # half (they are the same row range by construction).
        xTl = np.ascontiguousarray(x[b, h * SQ : (h + 1) * SQ, :].T).astype(bf16)
        # Partner-half row bases into the rank-ordered AllGather outputs.
        poff = np.array(
            [[(1 - h) * ET * P, (1 - h) * HKT * P]], dtype=np.uint32
        )
        in_maps.append(
            {
                "xT": xTl,
                "poff": poff,
                "wqT": wqT,
                "wkT": wkT,
                "wvT": wvT,
                "bq": bqp,
                "bk": bkp,
                "bv": bvp,
            }
        )
    return in_maps


def _run(in_maps, **kwargs):
    from concourse.bass_utils import run_bass_kernel_spmd

    nc = _get_nc()
    return run_bass_kernel_spmd(nc, in_maps, core_ids=list(range(NCORES)), **kwargs)


def kernel(x, Wq, bq, Wk, bk, Wv, bv):
    x = np.asarray(x, dtype=np.float32)
    Wq = np.asarray(Wq, dtype=np.float32)
    Wk = np.asarray(Wk, dtype=np.float32)
    Wv = np.asarray(Wv, dtype=np.float32)
    bq = np.asarray(bq, dtype=np.float32)
    bk = np.asarray(bk, dtype=np.float32)
    bv = np.asarray(bv, dtype=np.float32)

    res = _run(_make_in_maps(x, Wq, bq, Wk, bk, Wv, bv))
    out = np.empty((B, S, D), dtype=np.float32)
    for c in range(NCORES):
        b, h = divmod(c, 2)
        out[b, h * SQ : (h + 1) * SQ, :] = np.asarray(res.results[c]["ot"]).T
    return out


# revision 18
# speedup vs baseline: 1.0770x; 1.0770x over previous
"""BertSelfAttention (B=4, S=4096, D=512) on 8 TRN2 NeuronCores.

Sharding: core c handles batch b = c//2 and query-row half h = c%2
(2048 q rows). K/V are computed on-core for the full 4096 keys of that
batch (halves exchanged within each core pair), avoiding big collectives.

Layout trick: everything is computed transposed so no on-device
transposes are needed:
  QT[e, q] = Wq @ x.T          (lhsT = WqT chunks, rhs = xT chunks)
  KT[e, k] = Wk @ x.T
  V [k, e] = x @ Wv.T          (lhsT = xT chunks,  rhs = WvT)
  ST[k, q] = K Q.T             (lhsT = KT chunks,  rhs = QT)   -> exp -> PT
  OT[e, q] = V.T P.T           (lhsT = V chunks,   rhs = PT)
Softmax runs without max-subtraction (scores are ~N(0, 0.3^2), so exp
cannot overflow and the result is mathematically identical).

Precision: projections run in bf16 (fp32 PSUM); Q/K/V/P are quantized
to fp8e4 (e4m3) on PSUM evacuation and the two big matmuls (scores and
attn@V) run as fp8 DoubleRow matmuls: lhsT [128, 2, 128] and rhs
[128, 2, 512] slices pair two 128-deep contraction chunks per pass,
halving PE cycles per MAC. The dominant quantization-error term,
P @ (V - Vq), has a rank-1 component: softmax weights sum to exactly 1
(we normalize by the row sums of the quantized P), so the key-mean of
the V quantization error adds the same vector to every output row.
Each core accumulates column sums of (V_f32 - V_fp8) for its local
half on the otherwise-idle Vector engine, partition-reduces them with
a ones-matmul, and a tiny third AllGather ([1, 512] f32) shares them
across the pair; the correction folds into the bv bias applied during
output evacuation. Row sums of P are accumulated on the Vector engine
(one add per exp'd tile, pipelined behind the Scalar-engine exps),
then reduced across partitions and replicated by a single all-ones
matmul per q-chunk. Normalization + bias happen during OT evacuation
(exact: P@(V + 1*bv)/rowsum = P@V/rowsum + bv). A burst of throwaway
matmuls warms the PE HAM clock gate during the initial DMA wait, and
input DMAs are column-chunked and issued in first-consumer order on
the SP hardware queue (weights on the ACT queue) so the first
projection starts after ~1 MB of transfer.
"""

import sys

for _p in ("/opt/trn_rl_repo", "/root/.axon_site/_ro/trn_rl_repo"):
    if _p not in sys.path:
        sys.path.append(_p)

import numpy as np
import ml_dtypes

B, S, D = 4, 4096, 512
NCORES = 8
SQ = S // 2  # query rows per core
P = 128
NQ = 512  # q-chunk width (moving free dim)
DT = D // P  # 4 contraction chunks for d
ET = D // P  # 4 e tiles
KTI = S // P  # 32 k tiles
QC = SQ // NQ  # 4 q chunks per core
KC = S // NQ  # 8 k chunks (KT projection)
HKT = KTI // 2  # 16 local k-tiles per core
HS = S // 2  # 2048 local keys per core
SCALE = 1.0 / float(np.sqrt(np.float32(D)))

_CACHE = {}


def _split_excess_waits(nc, mybir, max_waits=1):
    """This walrus build rejects instructions carrying more than a couple of
    sync waits. Cap every instruction at `max_waits`, spilling the rest onto
    same-engine InstNoOps inserted immediately before it (equivalent
    semantics: the engine's stream stalls at the nop instead)."""
    for f in nc.m.functions:
        for bb in f.blocks:
            old = list(bb.instructions)
            new = []
            for inst in old:
                si = inst.sync_info
                waits = list(si.on_wait) if si is not None and si.on_wait else []
                if len(waits) > max_waits:
                    keep = waits[-max_waits:]
                    excess = waits[:-max_waits]
                    for i in range(0, len(excess), max_waits):
                        nop = mybir.InstNoOp(
                            name=f"waitnop-{nc.next_id()}", ins=[], outs=[]
                        )
                        nop.engine = inst.engine
                        nop.sync_info = mybir.SyncInfo(
                            on_wait=excess[i : i + max_waits], on_update=[]
                        )
                        new.append(nop)
                    inst.sync_info = mybir.SyncInfo(
                        on_wait=keep,
                        on_update=list(si.on_update) if si.on_update else [],
                    )
                new.append(inst)
            if len(new) != len(old):
                bb.instructions[:] = new


def _build_nc():
    import concourse.bass as bass
    import concourse.mybir as mybir
    import concourse.tile as tile
    from contextlib import ExitStack

    bf = mybir.dt.bfloat16
    f32 = mybir.dt.float32
    f8 = mybir.dt.float8e4
    AF = mybir.ActivationFunctionType
    DR = mybir.MatmulPerfMode.DoubleRow

    u32 = mybir.dt.uint32
    nc = bass.Bass()
    xT = nc.declare_dram_parameter("xT", [D, SQ], bf, isOutput=False)
    wqT = nc.declare_dram_parameter("wqT", [D, D], bf, isOutput=False)
    wkT = nc.declare_dram_parameter("wkT", [D, D], bf, isOutput=False)
    wvT = nc.declare_dram_parameter("wvT", [D, D], bf, isOutput=False)
    bqp = nc.declare_dram_parameter("bq", [P, ET], f32, isOutput=False)
    bkp = nc.declare_dram_parameter("bk", [P, ET], f32, isOutput=False)
    bvp = nc.declare_dram_parameter("bv", [P, ET], f32, isOutput=False)
    # Host-computed row bases into the AllGather outputs for the PARTNER
    # half (rank-dependent: (1-h)*512 + e*128 for KT, (1-h)*2048 + j*128
    # for V). Drives dynamic (register-offset) DMAs.
    poffp = nc.declare_dram_parameter("poff", [1, 2], u32, isOutput=False)
    ot = nc.declare_dram_parameter("ot", [D, SQ], f32, isOutput=True)

    with tile.TileContext(nc) as tc, ExitStack() as ctx:
        const_pool = ctx.enter_context(tc.tile_pool(name="const", bufs=1))
        persist = ctx.enter_context(tc.tile_pool(name="persist", bufs=1))
        outp = ctx.enter_context(tc.tile_pool(name="outp", bufs=2))

        ones = const_pool.tile([P, P], bf, tag="ones")
        nc.vector.memset(ones, 1.0)
        bq_sb = const_pool.tile([P, ET], f32, tag="bq")
        bk_sb = const_pool.tile([P, ET], f32, tag="bk")
        bv_sb = const_pool.tile([P, ET], f32, tag="bv")
        wq_sb = [const_pool.tile([P, D], bf, tag=f"wq{d}", name=f"wq{d}") for d in range(DT)]
        wk_sb = [const_pool.tile([P, D], bf, tag=f"wk{d}", name=f"wk{d}") for d in range(DT)]
        wv_sb = [const_pool.tile([P, D], bf, tag=f"wv{d}", name=f"wv{d}") for d in range(DT)]
        # fp8 operand tiles for the DoubleRow matmuls; contraction-paired
        # chunks live in dim 1 so [:, e:e+2, cols] is a valid 3D AP.
        qt_sb = persist.tile([P, ET, SQ], f8, tag="qt", name="qt")
        # K/V k-order per core: [my half, partner half]. Separate tiles per
        # half so partner DMA-writes create no false deps on local reads.
        kt_loc = persist.tile([P, ET, HS], f8, tag="ktl", name="ktl")
        kt_rem = persist.tile([P, ET, HS], f8, tag="ktr", name="ktr")
        v_loc = persist.tile([P, HKT, D], f8, tag="vl", name="vl")
        v_rem = persist.tile([P, HKT, D], f8, tag="vr", name="vr")
        poff_sb = const_pool.tile([1, 2], mybir.dt.uint32, tag="poff")
        bias2 = const_pool.tile([P, ET], f32, tag="bias2")

        # ---- Phase 1+2: load local x, project Q + local-half K/V,
        # AllGather the K/V halves within each core pair ----
        with (
            tc.tile_pool(name="xin", bufs=1) as xin_pool,
            tc.tile_pool(name="psA", bufs=4, space="PSUM") as psA,
            tc.tile_pool(name="dram", bufs=1, space="DRAM") as dram,
        ):
            ktl_d = dram.tile([ET * P, HS], f8, tag="ktl_d")
            ktg_d = dram.tile([2 * ET * P, HS], f8, tag="ktg_d")
            vl_d = dram.tile([HKT * P, D], f8, tag="vl_d")
            vg_d = dram.tile([2 * HKT * P, D], f8, tag="vg_d")
            sl_d = dram.tile([1, D], f32, tag="sl_d")
            sg_d = dram.tile([2, D], f32, tag="sg_d")

            x_sb = [xin_pool.tile([P, HS], bf, tag=f"x{d}", name=f"x{d}") for d in range(DT)]
            # Column-chunked loads in first-consumer order (SP HW queue);
            # weights ride the ACT HW queue in parallel.
            # One narrow chunk first (fast start for the first KT groups),
            # then the remaining columns in one wide DMA per d-tile to keep
            # SP-side issue overhead (~0.6us per DMA) off the critical path.
            for d in range(DT):
                nc.sync.dma_start(
                    out=x_sb[d][:, :NQ], in_=xT[d * P : (d + 1) * P, :NQ]
                )
            nc.sync.dma_start(out=bk_sb, in_=bkp[:, :])
            nc.sync.dma_start(out=bq_sb, in_=bqp[:, :])
            nc.sync.dma_start(out=bv_sb, in_=bvp[:, :])
            nc.sync.dma_start(out=poff_sb, in_=poffp[:, :])
            for kc in range(1, QC):
                for d in range(DT):
                    nc.sync.dma_start(
                        out=x_sb[d][:, kc * NQ : (kc + 1) * NQ],
                        in_=xT[d * P : (d + 1) * P, kc * NQ : (kc + 1) * NQ],
                    )
            for d in range(DT):
                nc.scalar.dma_start(out=wk_sb[d], in_=wkT[d * P : (d + 1) * P, :])
            for d in range(DT):
                nc.scalar.dma_start(out=wv_sb[d], in_=wvT[d * P : (d + 1) * P, :])
            for d in range(DT):
                nc.scalar.dma_start(out=wq_sb[d], in_=wqT[d * P : (d + 1) * P, :])

            # Warm the PE HAM clock gate (~3.4us of activity flips it from
            # 1.2 to 2.4 GHz) with throwaway matmuls while the first input
            # DMAs are still in flight.
            warm_ps = psA.tile([P, P], f32, tag="warm", name="warm_ps", bufs=1)
            for _ in range(96):
                nc.tensor.matmul(warm_ps, lhsT=ones, rhs=ones, start=True, stop=True)

            # KT local half [e, 0:2048] (bias bk fused on evacuation)
            for kc in range(QC):
                for e in range(ET):
                    ps = psA.tile([P, NQ], f32, tag="ps")
                    for d in range(DT):
                        nc.tensor.matmul(
                            ps,
                            lhsT=wk_sb[d][:, e * P : (e + 1) * P],
                            rhs=x_sb[d][:, kc * NQ : (kc + 1) * NQ],
                            start=(d == 0),
                            stop=(d == DT - 1),
                        )
                    nc.scalar.activation(
                        out=kt_loc[:, e, kc * NQ : (kc + 1) * NQ],
                        in_=ps,
                        func=AF.Identity,
                        bias=bk_sb[:, e : e + 1],
                        scale=1.0,
                    )
            for e in range(ET):
                nc.sync.dma_start(out=ktl_d[e * P : (e + 1) * P, :], in_=kt_loc[:, e, :])
            # Start the KT exchange immediately: the CC engine is idle and
            # the partner half gates the second half of the score phase.
            pairs = [[2 * i, 2 * i + 1] for i in range(NCORES // 2)]
            nc.gpsimd.collective_compute(
                "AllGather",
                mybir.AluOpType.bypass,
                replica_groups=pairs,
                ins=[ktl_d.opt()],
                outs=[ktg_d.opt()],
            )
            # V local half, tiles 0..15 (no bias; bv folded in at the end).
            # The Vector engine accumulates column sums of the fp8
            # quantization error for the rank-1 output correction.
            dacc = outp.tile([P, D], f32, tag="dacc", bufs=1)
            nc.vector.memset(dacc, 0.0)
            for k in range(HKT):
                ps = psA.tile([P, D], f32, tag="ps")
                for d in range(DT):
                    nc.tensor.matmul(
                        ps,
                        lhsT=x_sb[d][:, k * P : (k + 1) * P],
                        rhs=wv_sb[d][:, :],
                        start=(d == 0),
                        stop=(d == DT - 1),
                    )
                nc.scalar.copy(out=v_loc[:, k, :], in_=ps)
                dtmp = outp.tile([P, D], f32, tag="dtmp", bufs=2)
                nc.vector.tensor_sub(dtmp, ps, v_loc[:, k, :])
                nc.vector.tensor_add(dacc, dacc, dtmp)
                nc.sync.dma_start(out=vl_d[k * P : (k + 1) * P, :], in_=v_loc[:, k, :])
            # Partition-reduce the dV column sums and publish for the pair.
            dsum_bf = outp.tile([P, D], bf, tag="dsum_bf", bufs=1)
            nc.vector.tensor_copy(out=dsum_bf, in_=dacc)
            ps1 = psA.tile([1, D], f32, tag="ps1", bufs=1)
            nc.tensor.matmul(ps1, lhsT=ones[:, 0:1], rhs=dsum_bf, start=True, stop=True)
            sl_sb = outp.tile([1, D], f32, tag="sl_sb", bufs=1)
            nc.scalar.copy(out=sl_sb, in_=ps1)
            nc.sync.dma_start(out=sl_d, in_=sl_sb)

            # Exchange the V half + dV column sums while QT + scores run.
            nc.gpsimd.collective_compute(
                "AllGather",
                mybir.AluOpType.bypass,
                replica_groups=pairs,
                ins=[vl_d.opt()],
                outs=[vg_d.opt()],
            )
            nc.gpsimd.collective_compute(
                "AllGather",
                mybir.AluOpType.bypass,
                replica_groups=pairs,
                ins=[sl_d.opt()],
                outs=[sg_d.opt()],
            )

            # Partner-half loads from the gather outputs, issued before the
            # QT projection so the transfers start the moment each gather
            # lands. The row base is rank-dependent, supplied by the host
            # via `poff` and applied as a dynamic (register) offset.
            SP = [mybir.EngineType.SP]
            kt_base = nc.values_load(
                poff_sb[0:1, 0:1], engines=SP,
                min_val=0, max_val=ET * P,
                skip_runtime_bounds_check=True,
            )
            nc.sync.dma_start(
                out=kt_rem,
                in_=ktg_d[bass.ds(kt_base, ET * P), :].rearrange(
                    "(e p) c -> p e c", p=P
                ),
            )
            v_base = nc.values_load(
                poff_sb[0:1, 1:2], engines=SP,
                min_val=0, max_val=HKT * P,
                skip_runtime_bounds_check=True,
            )
            nc.sync.dma_start(
                out=v_rem,
                in_=vg_d[bass.ds(v_base, HKT * P), :].rearrange(
                    "(j p) c -> p j c", p=P
                ),
            )
            s_a = outp.tile([P, ET], f32, tag="s_a", bufs=1)
            s_b = outp.tile([P, ET], f32, tag="s_b", bufs=1)
            nc.sync.dma_start(
                out=s_a, in_=sg_d[0:1, :].rearrange("r (et p) -> (r p) et", p=P)
            )
            nc.sync.dma_start(
                out=s_b, in_=sg_d[1:2, :].rearrange("r (et p) -> (r p) et", p=P)
            )

            # QT[e, q] (bias bq fused on evacuation)
            for qc in range(QC):
                for e in range(ET):
                    ps = psA.tile([P, NQ], f32, tag="ps")
                    for d in range(DT):
                        nc.tensor.matmul(
                            ps,
                            lhsT=wq_sb[d][:, e * P : (e + 1) * P],
                            rhs=x_sb[d][:, qc * NQ : (qc + 1) * NQ],
                            start=(d == 0),
                            stop=(d == DT - 1),
                        )
                    nc.vector.tensor_scalar_add(
                        out=qt_sb[:, e, qc * NQ : (qc + 1) * NQ],
                        in0=ps,
                        scalar1=bq_sb[:, e : e + 1],
                    )

        # ---- Phase 3: attention ----
        # Static emission order staggers local-half score blocks ahead of
        # partner-half blocks so the PE has work while the AllGather +
        # partner DMAs are in flight.
        with (
            tc.tile_pool(name="pt", bufs=1) as pt_pool,
            tc.tile_pool(name="ps_st", bufs=3, space="PSUM") as ps_st,
            tc.tile_pool(name="ps_ot", bufs=2, space="PSUM") as ps_ot,
        ):
            ptl_tiles = {}
            ptp_tiles = {}
            rs_accs = {}
            rs_gps = {}

            def pt_slice(qc, k):
                if k < HKT:
                    return ptl_tiles[qc][:, k, :]
                return ptp_tiles[qc][:, k - HKT, :]

            def st_block(qc, k0, k1):
                qsl = slice(qc * NQ, (qc + 1) * NQ)
                if k0 == 0:
                    ptl_tiles[qc] = pt_pool.tile(
                        [P, HKT, NQ], f8, tag="ptl", name=f"ptl{qc}", bufs=4
                    )
                else:
                    ptp_tiles[qc] = pt_pool.tile(
                        [P, HKT, NQ], f8, tag="ptp", name=f"ptp{qc}", bufs=3
                    )
                # Two k-tiles share one 2-bank PSUM tile so each ACT Exp
                # covers 1024 columns, halving the per-instruction overhead.
                for k in range(k0, k1, 2):
                    ps = ps_st.tile([P, 2, NQ], f32, tag="st", name="st_ps")
                    for kh in range(2):
                        for e in (0, 2):
                            if k < HKT:
                                lhsT = kt_loc[:, e : e + 2, (k + kh) * P : (k + kh + 1) * P]
                            else:
                                kk = k + kh - HKT
                                lhsT = kt_rem[:, e : e + 2, kk * P : (kk + 1) * P]
                            nc.tensor.matmul(
                                ps[:, kh, :],
                                lhsT=lhsT,
                                rhs=qt_sb[:, e : e + 2, qsl],
                                start=(e == 0),
                                stop=(e == 2),
                                perf_mode=DR,
                            )
                    if k < HKT:
                        pt_pair = ptl_tiles[qc][:, k : k + 2, :]
                    else:
                        pt_pair = ptp_tiles[qc][:, k - HKT : k - HKT + 2, :]
                    nc.scalar.activation(
                        out=pt_pair, in_=ps, func=AF.Exp, scale=SCALE
                    )
                    if k == 0:
                        rs_accs[qc] = outp.tile(
                            [P, NQ], f32, tag="rs_acc", name=f"rs_acc{qc}", bufs=4
                        )
                        nc.vector.tensor_copy(out=rs_accs[qc], in_=pt_slice(qc, 0))
                    else:
                        nc.vector.tensor_add(
                            rs_accs[qc], rs_accs[qc], pt_slice(qc, k)
                        )
                    nc.vector.tensor_add(
                        rs_accs[qc], rs_accs[qc], pt_slice(qc, k + 1)
                    )

            recips = {}

            def bias2_compute():
                # dV column sums from both halves -> output bias correction
                # bias2 = bv + (s_local + s_partner) / S. Rank order in sg_d
                # is irrelevant since both rows are summed. Emitted here (at
                # first consumption) so the in-order ACT stream never stalls
                # on the small s-gather.
                nc.vector.tensor_add(s_a, s_a, s_b)
                for et in range(ET):
                    nc.scalar.activation(
                        out=bias2[:, et : et + 1],
                        in_=s_a[:, et : et + 1],
                        func=AF.Identity,
                        bias=bv_sb[:, et : et + 1],
                        scale=1.0 / S,
                    )

            rs_bfs = {}

            def finish_rsbf(qc):
                # ACT-side bf16 copy of the rowsum partials, emitted right
                # after st_block(qc, partner) so it sits directly behind that
                # block's exps in the in-order ACT stream.
                rs_bfs[qc] = outp.tile(
                    [P, NQ], bf, tag="rs_bf", bufs=2, name=f"rs_bf{qc}"
                )
                nc.scalar.copy(out=rs_bfs[qc], in_=rs_accs[qc])

            def finish_rsmm(qc):
                # Partition-reduce + replicate the rowsum partials with one
                # all-ones matmul. Deferred one PE block after finish_rsbf so
                # the PE never leads the ACT copy it depends on.
                rs_ps = ps_ot.tile([P, NQ], f32, tag="ot", name="rs_ps")
                nc.tensor.matmul(rs_ps, lhsT=ones, rhs=rs_bfs[qc], start=True, stop=True)
                recips[qc] = outp.tile([P, NQ], f32, tag="recip", bufs=2, name=f"recip{qc}")
                nc.vector.reciprocal(recips[qc], rs_ps)

            def finish_av(qc):
                qsl = slice(qc * NQ, (qc + 1) * NQ)
                recip = recips[qc]
                for e in range(ET):
                    ops = ps_ot.tile([P, NQ], f32, tag="ot")
                    for k in range(0, KTI, 2):
                        if k < HKT:
                            vlhsT = v_loc[:, k : k + 2, e * P : (e + 1) * P]
                            prhs = ptl_tiles[qc][:, k : k + 2, :]
                        else:
                            kk = k - HKT
                            vlhsT = v_rem[:, kk : kk + 2, e * P : (e + 1) * P]
                            prhs = ptp_tiles[qc][:, kk : kk + 2, :]
                        nc.tensor.matmul(
                            ops,
                            lhsT=vlhsT,
                            rhs=prhs,
                            start=(k == 0),
                            stop=(k == KTI - 2),
                            perf_mode=DR,
                        )
                    tmp = outp.tile([P, NQ], f32, tag="tmp", bufs=3)
                    nc.vector.tensor_mul(tmp, ops, recip)
                    nc.scalar.activation(
                        out=tmp,
                        in_=tmp,
                        func=AF.Identity,
                        bias=bias2[:, e : e + 1],
                        scale=1.0,
                    )
                    nc.sync.dma_start(out=ot[e * P : (e + 1) * P, qsl], in_=tmp)

            # Interleave: each q-chunk's rowsum reduction is emitted right
            # after its last score block (ahead of the next block's exps in
            # the ACT stream), and its AV matmuls run while the following
            # block's exp/rowsum tail resolves on ACT/DVE.
            st_block(0, 0, HKT)
            st_block(1, 0, HKT)
            st_block(2, 0, HKT)
            st_block(3, 0, HKT)
            st_block(0, HKT, KTI)
            finish_rsbf(0)
            st_block(1, HKT, KTI)
            finish_rsbf(1)
            finish_rsmm(0)
            bias2_compute()
            finish_av(0)
            st_block(2, HKT, KTI)
            finish_rsbf(2)
            finish_rsmm(1)
            finish_av(1)
            st_block(3, HKT, KTI)
            finish_rsbf(3)
            finish_rsmm(2)
            finish_av(2)
            finish_rsmm(3)
            finish_av(3)

    _split_excess_waits(nc, mybir)
    return nc


def _get_nc():
    if "nc" not in _CACHE:
        _CACHE["nc"] = _build_nc()
    return _CACHE["nc"]


def _make_in_maps(x, Wq, bq, Wk, bk, Wv, bv):
    bf16 = ml_dtypes.bfloat16
    wqT = np.ascontiguousarray(Wq.T).astype(bf16)
    wkT = np.ascontiguousarray(Wk.T).astype(bf16)
    wvT = np.ascontiguousarray(Wv.T).astype(bf16)
    bqp = np.ascontiguousarray(bq.reshape(ET, P).T).astype(np.float32)
    bkp = np.ascontiguousarray(bk.reshape(ET, P).T).astype(np.float32)
    bvp = np.ascontiguousarray(bv.reshape(ET, P).T).astype(np.float32)
    in_maps = []
    for c in range(NCORES):
        b, h = divmod(c, 2)
        # Local half of x[b].T: both this core's query columns and its K/V
        # half (they are the same row range by construction).
        xTl = np.ascontiguousarray(x[b, h * SQ : (h + 1) * SQ, :].T).astype(bf16)
        # Partner-half row bases into the rank-ordered AllGather outputs.
        poff = np.array(
            [[(1 - h) * ET * P, (1 - h) * HKT * P]], dtype=np.uint32
        )
        in_maps.append(
            {
                "xT": xTl,
                "poff": poff,
                "wqT": wqT,
                "wkT": wkT,
                "wvT": wvT,
                "bq": bqp,
                "bk": bkp,
                "bv": bvp,
            }
        )
    return in_maps


def _run(in_maps, **kwargs):
    from concourse.bass_utils import run_bass_kernel_spmd

    nc = _get_nc()
    return run_bass_kernel_spmd(nc, in_maps, core_ids=list(range(NCORES)), **kwargs)


def kernel(x, Wq, bq, Wk, bk, Wv, bv):
    x = np.asarray(x, dtype=np.float32)
    Wq = np.asarray(Wq, dtype=np.float32)
    Wk = np.asarray(Wk, dtype=np.float32)
    Wv = np.asarray(Wv, dtype=np.float32)
    bq = np.asarray(bq, dtype=np.float32)
    bk = np.asarray(bk, dtype=np.float32)
    bv = np.asarray(bv, dtype=np.float32)

    res = _run(_make_in_maps(x, Wq, bq, Wk, bk, Wv, bv))
    out = np.empty((B, S, D), dtype=np.float32)
    for c in range(NCORES):
        b, h = divmod(c, 2)
        out[b, h * SQ : (h + 1) * SQ, :] = np.asarray(res.results[c]["ot"]).T
    return out
